# revision 21
# baseline (speedup 1.0000x reference)
"""Trainium2 Bass kernel for nn_Attention_msa_TwoStream (sparse cosine attention).

Outputs depend only on v_cls (the qkv MLP / q / k path is dead code w.r.t. the
returned tensors).  The computation is:

    v   = v_cls viewed per-head                         (H=8, N=3200, d=64)
    vn  = v / (||v||_head + eps)
    S_h = vn_h @ vn_h^T          (per-head cosine sim)
    attn_h = S_h * keep_mask     (block mask: row j zeroes cols [bs,bs+9) except diag)
    x   = concat_h(attn_h @ v_h)                         -> (N, 512)
    x_out = [x, v_cls]                                   -> (1, N, 1024)
    raw_mean  = mean_h S_h ; sim_mask = raw_mean > 0.75
    sim_attn  = mean_h attn_h
    sim_round2 = renorm(sim_mask * softmax(sim_attn))    -> (N, N)

Sharding: rows (block-aligned, 400 rows/core on 8 cores) rather than heads, so
the head-sum needed by sim_round2 is a local K=512 matmul (PSUM accumulation)
and no collective is needed.  Per-core column coordinates are rotated by the
core's row offset so the block-diagonal mask window sits at a core-independent
local position (one uniform SPMD program; host un-rotates the sim output).

The masked attention x is decomposed exactly as
    x_h = vn_h @ G_h - (D_h ⊙ Z) @ v_h ,   G_h = vn_h^T v_h (global 64x64),
where D_h is the 100x100 block-diagonal window of S_h and Z the strictly
in-block zero pattern; this avoids materializing per-head NxN matrices.
"""

import os
import sys
from contextlib import ExitStack

for _p in ("/opt/trn_rl_repo", os.path.expanduser("~/.axon_site/_ro/trn_rl_repo")):
    if os.path.isdir(_p) and _p not in sys.path:
        sys.path.insert(0, _p)

import ml_dtypes
import numpy as np

import concourse.bass as bass
import concourse.tile as tile
from concourse import mybir
from concourse import bass2jax

N = 3200
C = 512
H = 8
D = 64
BLOCK = 10
EPS = 1e-8
THRESH = 0.75
NEG = -30.0

NCORES = 8
RPC = N // NCORES          # 400 rows per core
MT = 100                   # row tile (block-aligned, 4 tiles/core)
NTILES = RPC // MT
COL_TILES = [512] * 6 + [128]   # 3200 columns
BF16 = ml_dtypes.bfloat16

_PROG = None
LAST_RESULTS = None        # BassKernelResults of the last run (for profiling)


class _ChunkedDrainTC(tile.TileContext):
    """The walrus build here allows only one embedded sem-wait per NOP/Drain
    instruction; split the kernel-tail drain into single-wait drains."""

    def _drain_and_barrier(self, tick_clock, wait_clock):
        from concourse.vector_clock import ScopedClock, VectorClock

        gc = tick_clock.global_clock
        vec = list(gc)
        for i, v in enumerate(vec):
            if v <= 0:
                continue
            partial = [0] * len(vec)
            partial[i] = v
            d = self.nc.sync.drain()
            wait_clock.add_sem_waits(d.ins, ScopedClock({None: VectorClock(partial)}))
        self.nc.all_engine_barrier()
        assert self.sems is not None
        popped = self.nc._tile_sem_poison_stack.pop()
        assert popped is self._sem_poison
        self.nc.clear_and_free_semaphores(list(self.sems.allocated().values()))
        self.nc.all_engine_barrier()


def _build_program():
    nc = bass.Bass()
    dt = mybir.dt

    rot = nc.declare_dram_parameter("rot_vnt", [C, N], dt.bfloat16, isOutput=False)
    loc = nc.declare_dram_parameter("vnt_loc", [C, RPC], dt.float32, isOutput=False)
    vr = nc.declare_dram_parameter("v_rows", [RPC, C], dt.bfloat16, isOutput=False)
    gsb = nc.declare_dram_parameter("g_sb", [128, 4 * D], dt.float32, isOutput=False)
    kp = nc.declare_dram_parameter("keep100", [MT, MT], dt.float32, isOutput=False)
    zn = nc.declare_dram_parameter("zneg4", [MT, 512], dt.float32, isOutput=False)
    simo = nc.declare_dram_parameter("sim_rows", [RPC, N], dt.float32, isOutput=True)
    xo = nc.declare_dram_parameter("x_rows", [RPC, C], dt.float32, isOutput=True)

    with _ChunkedDrainTC(nc) as tc, ExitStack() as ctx:
        const = ctx.enter_context(tc.tile_pool(name="const", bufs=1))
        raw_ps = ctx.enter_context(tc.tile_pool(name="raw_ps", bufs=3, space="PSUM"))
        d_ps = ctx.enter_context(tc.tile_pool(name="d_ps", bufs=2, space="PSUM"))
        x_ps = ctx.enter_context(tc.tile_pool(name="x_ps", bufs=2, space="PSUM"))
        tpool = ctx.enter_context(tc.tile_pool(name="trow", bufs=4))
        mpool = ctx.enter_context(tc.tile_pool(name="m30", bufs=2))
        ppool = ctx.enter_context(tc.tile_pool(name="prob", bufs=4))
        dzpool = ctx.enter_context(tc.tile_pool(name="dz", bufs=4))
        xsb = ctx.enter_context(tc.tile_pool(name="xsb", bufs=4))
        spool = ctx.enter_context(tc.tile_pool(name="small", bufs=8))

        # --- preload constants ---
        rotc = []
        for k in range(4):
            t = const.tile([128, N], dt.bfloat16, tag=f"rotc{k}")
            nc.sync.dma_start(t[:], rot[128 * k:128 * (k + 1), :])
            rotc.append(t)
        locc = []
        for k in range(4):
            t = const.tile([128, RPC], dt.float32, tag=f"locc{k}")
            nc.sync.dma_start(t[:], loc[128 * k:128 * (k + 1), :])
            locc.append(t)
        vrt = []
        for t_i in range(NTILES):
            t = const.tile([MT, C], dt.bfloat16, tag=f"vrt{t_i}")
            nc.sync.dma_start(t[:], vr[MT * t_i:MT * (t_i + 1), :])
            vrt.append(t)
        gt = const.tile([128, 4 * D], dt.float32, tag="gt")
        nc.sync.dma_start(gt[:], gsb[:])
        # matmul operands must start at partition 0 on this runtime (base-64
        # APs fail at execution), so odd heads get their own partition-0 copies
        # loaded straight from DRAM.
        rot_odd, locc_odd, g_odd = [], [], []
        for io in range(4):
            h = 2 * io + 1
            t = const.tile([64, MT * NTILES + 28], dt.bfloat16, tag=f"rot_odd{io}")
            nc.sync.dma_start(t[:], rot[64 * h:64 * h + 64, :MT * NTILES + 28])
            rot_odd.append(t)
            t = const.tile([64, RPC], dt.float32, tag=f"locc_odd{io}")
            nc.sync.dma_start(t[:], loc[64 * h:64 * h + 64, :])
            locc_odd.append(t)
            t = const.tile([64, D], dt.float32, tag=f"g_odd{io}")
            nc.sync.dma_start(t[:], gsb[64:128, D * io:D * (io + 1)])
            g_odd.append(t)
        kpt = const.tile([MT, MT], dt.float32, tag="kpt")
        nc.sync.dma_start(kpt[:], kp[:])
        znt = const.tile([MT, 512], dt.float32, tag="znt")
        nc.sync.dma_start(znt[:], zn[:])
        # tiny DVE reads so later DVE ops don't need a second (DMA) wait slot
        scr = spool.tile([1, 1], dt.float32, tag="scr")
        nc.vector.tensor_copy(scr[:], znt[0:1, 0:1])
        scr2 = spool.tile([1, 1], dt.float32, tag="scr2")
        nc.vector.tensor_copy(scr2[:], kpt[0:1, 0:1])
        # tiny PE touch-matmuls likewise absorb each const tile's DMA wait
        # (instructions on this runtime have a single embedded wait slot)
        pe_consts = rotc + locc + vrt + rot_odd + locc_odd + g_odd + [gt]
        tch_ps = ctx.enter_context(tc.tile_pool(name="tch", bufs=1, space="PSUM"))
        tch = tch_ps.tile([1, 1], dt.float32, tag="tch")
        for ct in pe_consts:
            nc.tensor.matmul(tch[:], ct[0:1, 0:1], ct[0:1, 0:1],
                             start=True, stop=True)

        for ti in range(NTILES):
            T = MT * ti
            # ---- big head-summed similarity: raw[p, m] = sum_c vn[T+p, c] vn[m, c]
            trow = tpool.tile([MT, N], dt.float32)
            col0 = 0
            for w in COL_TILES:
                ps = raw_ps.tile([MT, 512], dt.float32)
                for k in range(4):
                    nc.tensor.matmul(
                        ps[:, :w],
                        rotc[k][:, T:T + MT],
                        rotc[k][:, col0:col0 + w],
                        start=(k == 0),
                        stop=(k == 3),
                    )
                # t = raw/8 (raw_mean units)
                nc.scalar.mul(trow[:, col0:col0 + w], ps[:, :w], 0.125)
                col0 += w

            # ---- per-head local (block-window) sims D_h, masked -> bf16 lhsT
            dzs = []
            for g2 in range(2):
                dps = d_ps.tile([MT, 512], dt.float32)
                for jj in range(4):
                    h = 4 * g2 + jj
                    if h % 2 == 0:
                        la = rotc[h // 2][0:64, T:T + MT]
                        ra = rotc[h // 2][0:64, T:T + 128]
                    else:
                        la = rot_odd[h // 2][:, T:T + MT]
                        ra = rot_odd[h // 2][:, T:T + 128]
                    nc.tensor.matmul(
                        dps[:, 128 * jj:128 * jj + 128], la, ra,
                        start=True,
                        stop=True,
                    )
                dz = dzpool.tile([MT, 512], dt.bfloat16)
                for jj in range(4):
                    s = slice(128 * jj, 128 * jj + 128)
                    nc.vector.tensor_mul(dz[:, s], dps[:, s], znt[:, s])
                dzs.append(dz)

            # ---- x = vn @ G  -  (D ⊙ Z) @ v   (accumulated in PSUM per head)
            xps = x_ps.tile([MT, C], dt.float32)
            for h in range(H):
                k = h // 2
                # mm order: correction matmul first so the slot-release and
                # dz dependencies merge into a single DVE wait
                nc.tensor.matmul(
                    xps[:, D * h:D * (h + 1)],
                    dzs[h // 4][:, 128 * (h % 4):128 * (h % 4) + MT],
                    vrt[ti][:, D * h:D * (h + 1)],
                    start=True,
                    stop=False,
                )
                if h % 2 == 0:
                    la = locc[k][0:64, T:T + MT]
                    ra = gt[0:64, D * k:D * (k + 1)]
                else:
                    la = locc_odd[k][:, T:T + MT]
                    ra = g_odd[k][:, :]
                nc.tensor.matmul(
                    xps[:, D * h:D * (h + 1)], la, ra,
                    start=False,
                    stop=True,
                )
            xst = xsb.tile([MT, C], dt.float32)
            nc.vector.tensor_copy(xst[:], xps[:])
            nc.gpsimd.dma_start(xo[T:T + MT, :], xst[:])

            # ---- masked softmax + renormalization (rows of sim_round2)
            m30 = mpool.tile([MT, N], dt.float32)
            # m30 = -30 where raw_mean <= thresh (sim_mask complement), else 0
            nc.vector.tensor_scalar(
                m30[:], trow[:], THRESH, NEG,
                mybir.AluOpType.is_le, mybir.AluOpType.mult,
            )
            # apply block keep-mask to the local diagonal window
            nc.vector.tensor_mul(trow[:, T:T + MT], trow[:, T:T + MT], kpt[:])
            # u = sim_attn + mask-penalty
            nc.vector.tensor_add(m30[:], m30[:], trow[:])
            prob = ppool.tile([MT, N], dt.float32)
            sp = spool.tile([MT, 1], dt.float32)
            nc.scalar.activation(
                prob[:], m30[:], mybir.ActivationFunctionType.Exp, accum_out=sp[:]
            )
            rp = spool.tile([MT, 1], dt.float32)
            nc.vector.reciprocal(rp[:], sp[:])
            nc.vector.tensor_scalar_mul(prob[:], prob[:], rp[:])
            nc.gpsimd.dma_start(simo[T:T + MT, :], prob[:])

    return nc


def _host_prep(v_cls):
    v = np.ascontiguousarray(v_cls[0]).astype(np.float32)     # (N, C)
    vh = v.reshape(N, H, D)
    nrm = np.linalg.norm(vh, axis=2, keepdims=True)
    vn = (vh / (nrm + np.float32(EPS))).reshape(N, C).astype(np.float32)
    vnT = np.ascontiguousarray(vn.T)                          # (C, N) f32

    g_all = np.empty((C, D), np.float32)
    for h in range(H):
        g_all[D * h:D * (h + 1)] = (
            vn[:, D * h:D * (h + 1)].astype(np.float64).T
            @ v[:, D * h:D * (h + 1)].astype(np.float64)
        ).astype(np.float32)
    # chunk-packed: g_sb[p, k*D + n] = g_all[128k + p, n]
    g_sb = np.ascontiguousarray(
        g_all.reshape(4, 128, D).transpose(1, 0, 2).reshape(128, 4 * D)
    )

    a = np.arange(MT)
    same_blk = (a[:, None] // BLOCK) == (a[None, :] // BLOCK)
    # zneg4[a, b]: a = source row q (partition), b = dest row p (free)
    zneg = -(same_blk & (a[:, None] % BLOCK <= BLOCK - 2)
             & (a[:, None] != a[None, :])).astype(np.float32)
    zneg4 = np.zeros((MT, 512), np.float32)
    for jj in range(4):
        zneg4[:, 128 * jj:128 * jj + MT] = zneg
    # keep100[p, q]: zero where column q is in p's zero-zone (except diagonal)
    keep100 = 1.0 - (same_blk & (a[None, :] % BLOCK <= BLOCK - 2)
                     & (a[None, :] != a[:, None])).astype(np.float32)

    vnT_bf = vnT.astype(BF16)
    in_maps = []
    for c in range(NCORES):
        r0 = RPC * c
        in_maps.append({
            "rot_vnt": np.ascontiguousarray(np.roll(vnT_bf, -r0, axis=1)),
            "vnt_loc": np.ascontiguousarray(vnT[:, r0:r0 + RPC]),
            "v_rows": np.ascontiguousarray(v[r0:r0 + RPC]).astype(BF16),
            "g_sb": g_sb,
            "keep100": keep100,
            "zneg4": zneg4,
        })
    return v, in_maps


_RUNNER = None


def _make_runner():
    """Build the SPMD jitted executable once (mirrors bass2jax.run_bass_via_pjrt,
    without output-buffer donation so the callable can be re-invoked for timing)."""
    import jax
    from jax.sharding import Mesh, PartitionSpec
    try:
        from jax.experimental.shard_map import shard_map
    except ImportError:
        shard_map = jax.shard_map

    nc = _build_program()
    bass2jax.install_neuronx_cc_hook()

    partition_name = nc.partition_id_tensor.name if nc.partition_id_tensor else None
    in_names, out_names, out_avals, zero_outs = [], [], [], []
    for alloc in nc.m.functions[0].allocations:
        if not isinstance(alloc, mybir.MemoryLocationSet):
            continue
        name = alloc.memorylocations[0].name
        if alloc.kind == "ExternalInput":
            if name != partition_name:
                in_names.append(name)
        elif alloc.kind == "ExternalOutput":
            shape = tuple(alloc.tensor_shape)
            dtype = mybir.dt.np(alloc.dtype)
            out_names.append(name)
            out_avals.append(jax.core.ShapedArray(shape, dtype))
            zero_outs.append(np.zeros(shape, dtype))
    n_params = len(in_names)
    all_names = in_names + out_names
    if partition_name is not None:
        all_names = all_names + [partition_name]

    def _body(*args):
        operands = list(args)
        if partition_name is not None:
            operands.append(bass2jax.partition_id_tensor())
        outs = bass2jax._bass_exec_p.bind(
            *operands,
            out_avals=tuple(out_avals),
            in_names=tuple(all_names),
            out_names=tuple(out_names),
            lowering_input_output_aliases=(),
            sim_require_finite=True,
            sim_require_nnan=True,
            nc=nc,
        )
        return tuple(outs)

    devices = jax.devices()[:NCORES]
    mesh = Mesh(np.asarray(devices), ("core",))
    n_all = n_params + len(out_names)
    donate = tuple(range(n_params, n_all))
    fn = jax.jit(
        shard_map(
            _body, mesh=mesh,
            in_specs=(PartitionSpec("core"),) * n_all,
            out_specs=(PartitionSpec("core"),) * len(out_names),
            check_rep=False,
        ),
        donate_argnums=donate,
        keep_unused=True,
    )
    return fn, in_names, out_names, zero_outs, mesh


def _get_runner():
    global _RUNNER
    if _RUNNER is None:
        _RUNNER = _make_runner()
    return _RUNNER


def _device_inputs(in_maps):
    fn, in_names, out_names, zero_outs, _mesh = _get_runner()
    concat_in = [
        np.concatenate([m[name] for m in in_maps], axis=0) for name in in_names
    ]
    concat_zeros = [
        np.zeros((NCORES * z.shape[0], *z.shape[1:]), z.dtype) for z in zero_outs
    ]
    return concat_in + concat_zeros


def _run_device(args):
    fn = _get_runner()[0]
    out = fn(*args)
    return [np.asarray(o) for o in out]


def _assemble(outs, v_cls):
    # outs order follows out_names declaration order: sim_rows, x_rows
    sim_g = outs[0].reshape(NCORES, RPC, N)
    x_g = outs[1].reshape(NCORES, RPC, C)
    sim = np.concatenate(
        [np.roll(sim_g[c], RPC * c, axis=1) for c in range(NCORES)], 0)
    x = x_g.reshape(N, C)
    x_out = np.concatenate([x.reshape(1, N, C), v_cls], axis=-1).astype(np.float32)
    return x_out, np.ascontiguousarray(sim, dtype=np.float32)


def kernel(**inputs):
    v_cls = np.asarray(inputs["v_cls"], dtype=np.float32)
    v, in_maps = _host_prep(v_cls)
    args = _device_inputs(in_maps)
    outs = _run_device(args)
    return _assemble(outs, v_cls)


def bench_ns(inputs, warmup=2, iters=10):
    """Steady-state per-invocation device time in ns (inputs device-resident)."""
    import time
    import jax
    v_cls = np.asarray(inputs["v_cls"], dtype=np.float32)
    _, in_maps = _host_prep(v_cls)
    from jax.sharding import NamedSharding, PartitionSpec
    fn, in_names, _, _, mesh = _get_runner()
    sh = NamedSharding(mesh, PartitionSpec("core"))
    all_args = _device_inputs(in_maps)
    n_params = len(in_names)
    ins = [jax.device_put(a, sh) for a in all_args[:n_params]]
    # outputs are donated; ping-pong the previous iteration's outputs back in
    # (the kernel writes every element, so initial contents don't matter)
    outs = [jax.device_put(a, sh) for a in all_args[n_params:]]
    for _ in range(warmup):
        outs = list(fn(*ins, *outs))
    jax.block_until_ready(outs)
    t0 = time.perf_counter()
    for _ in range(iters):
        outs = list(fn(*ins, *outs))
    jax.block_until_ready(outs)
    t1 = time.perf_counter()
    return (t1 - t0) / iters * 1e9
